# revision 1
# baseline (speedup 1.0000x reference)
"""Data-parallel ActorCritic inference across 8 NeuronCores.

Shards obs along batch (2048 samples/core), replicates all weights
(<1 MB), runs the full preprocess+CNN+MLP network per shard, and
concatenates the per-core outputs. Self-contained: all shapes are
hardcoded to the problem spec (obs [16384, 490] fp32).
"""
import numpy as np
import jax
import jax.numpy as jnp
from jax import lax
from functools import partial

GRID = 16
N_UNITS = 20
GRID_OBS_SIZE = GRID * GRID + N_UNITS * 10 + 2  # 458
N_CORES = 8


def _preprocess(obs):
    Bn = obs.shape[0]
    terrain = obs[:, :GRID * GRID].reshape(Bn, 1, GRID, GRID)
    units = obs[:, GRID * GRID:GRID * GRID + N_UNITS * 10].reshape(Bn, N_UNITS, 10)
    unit_obs = obs[:, GRID_OBS_SIZE:]
    team = units[..., 1]
    r = (units[..., 2] * (GRID - 1)).astype(jnp.int32)
    c = (units[..., 3] * (GRID - 1)).astype(jnp.int32)
    hp = units[..., 4]
    moved = units[..., 5]
    acted = units[..., 6]
    alive = hp > 0
    blue = team < 0.5
    flat = r * GRID + c
    bidx = jnp.arange(Bn)[:, None]

    def scat(vals, mask):
        z = jnp.zeros((Bn, GRID * GRID), obs.dtype)
        z = z.at[bidx, flat].add(jnp.where(mask, vals, 0.0))
        return z.reshape(Bn, 1, GRID, GRID)

    ones = jnp.ones_like(hp)
    mb = alive & blue
    mr = alive & ~blue
    grid = jnp.concatenate([
        terrain,
        scat(ones, mb), scat(ones, mr),
        scat(hp, mb), scat(hp, mr),
        scat(moved, mb), scat(acted, mb),
        scat(moved, mr), scat(acted, mr),
    ], axis=1)
    return grid, unit_obs


def _conv(x, w, b):
    y = lax.conv_general_dilated(x, w, (1, 1), 'SAME',
                                 dimension_numbers=('NCHW', 'OIHW', 'NCHW'))
    return jax.nn.relu(y + b[None, :, None, None])


def _lin(x, w, b):
    return x @ w.T + b


def _forward(obs, conv1_w, conv1_b, conv2_w, conv2_b, conv3_w, conv3_b,
             fc_w, fc_b, u1_w, u1_b, u2_w, u2_b, t1_w, t1_b, t2_w, t2_b,
             actor_w, actor_b, critic_w, critic_b):
    grid, unit = _preprocess(obs)
    x = _conv(grid, conv1_w, conv1_b)
    x = _conv(x, conv2_w, conv2_b)
    x = _conv(x, conv3_w, conv3_b)
    Bn = x.shape[0]
    x = x.reshape(Bn, 64, 4, 4, 4, 4).mean(axis=(3, 5))
    x = x.reshape(Bn, 64 * 4 * 4)
    cnn_feat = jax.nn.relu(_lin(x, fc_w, fc_b))
    uh = jax.nn.relu(_lin(unit, u1_w, u1_b))
    unit_feat = jax.nn.relu(_lin(uh, u2_w, u2_b))
    t = jnp.concatenate([cnn_feat, unit_feat], axis=-1)
    t = jax.nn.relu(_lin(t, t1_w, t1_b))
    t = jax.nn.relu(_lin(t, t2_w, t2_b))
    logits = _lin(t, actor_w, actor_b)
    value = _lin(t, critic_w, critic_b)
    return logits, value


_WEIGHT_KEYS = [
    "conv1_w", "conv1_b", "conv2_w", "conv2_b", "conv3_w", "conv3_b",
    "fc_w", "fc_b", "u1_w", "u1_b", "u2_w", "u2_b", "t1_w", "t1_b",
    "t2_w", "t2_b", "actor_w", "actor_b", "critic_w", "critic_b",
]

_pmapped = None


def _get_pmapped():
    global _pmapped
    if _pmapped is None:
        devs = jax.devices()[:N_CORES]
        _pmapped = jax.pmap(
            _forward,
            in_axes=(0,) + (None,) * len(_WEIGHT_KEYS),
            devices=devs,
        )
    return _pmapped


def kernel(**inputs):
    obs = np.asarray(inputs["obs"], dtype=np.float32)
    B = obs.shape[0]
    shard = B // N_CORES
    obs_sharded = obs.reshape(N_CORES, shard, obs.shape[1])
    weights = [np.asarray(inputs[k], dtype=np.float32) for k in _WEIGHT_KEYS]
    fn = _get_pmapped()
    logits, value = fn(obs_sharded, *weights)
    logits = np.asarray(logits).reshape(B, -1)
    value = np.asarray(value).reshape(B, -1)
    return logits.astype(np.float32), value.astype(np.float32)


# revision 4
# speedup vs baseline: 1.0490x; 1.0490x over previous
"""Data-parallel ActorCritic inference across 8 NeuronCores.

Shards obs along batch (2048 samples/core), replicates all weights
(<1 MB), runs the full preprocess+CNN+MLP network per shard, and
concatenates the per-core outputs. Self-contained: all shapes are
hardcoded to the problem spec (obs [16384, 490] fp32).
"""
import numpy as np
import jax
import jax.numpy as jnp
from jax import lax
from functools import partial

GRID = 16
N_UNITS = 20
GRID_OBS_SIZE = GRID * GRID + N_UNITS * 10 + 2  # 458
N_CORES = 8


def _preprocess(obs):
    Bn = obs.shape[0]
    terrain = obs[:, :GRID * GRID].reshape(Bn, 1, GRID, GRID)
    units = obs[:, GRID * GRID:GRID * GRID + N_UNITS * 10].reshape(Bn, N_UNITS, 10)
    unit_obs = obs[:, GRID_OBS_SIZE:]
    team = units[..., 1]
    r = (units[..., 2] * (GRID - 1)).astype(jnp.int32)
    c = (units[..., 3] * (GRID - 1)).astype(jnp.int32)
    hp = units[..., 4]
    moved = units[..., 5]
    acted = units[..., 6]
    alive = hp > 0
    blue = team < 0.5
    flat = r * GRID + c
    bidx = jnp.arange(Bn)[:, None]

    # Dense one-hot matmul scatter: distinct cells per sample guarantee
    # sum == set semantics. onehot: (B, U, P) in bf16; vals: (B, 8, U).
    onehot = (flat[:, :, None] == jnp.arange(GRID * GRID)[None, None, :])
    onehot = onehot.astype(jnp.bfloat16)
    ones = jnp.ones_like(hp)
    mb = (alive & blue).astype(obs.dtype)
    mr = (alive & ~blue).astype(obs.dtype)
    vals = jnp.stack([
        ones * mb, ones * mr,
        hp * mb, hp * mr,
        moved * mb, acted * mb,
        moved * mr, acted * mr,
    ], axis=1).astype(jnp.bfloat16)                    # (B, 8, U)
    chans = jnp.einsum('bcu,bup->bcp', vals, onehot,
                       preferred_element_type=jnp.float32)
    chans = chans.reshape(Bn, 8, GRID, GRID).astype(obs.dtype)
    grid = jnp.concatenate([terrain, chans], axis=1)
    return grid, unit_obs


def _conv(x, w, b):
    y = lax.conv_general_dilated(
        x.astype(jnp.bfloat16), w.astype(jnp.bfloat16), (1, 1), 'SAME',
        dimension_numbers=('NCHW', 'OIHW', 'NCHW'),
        preferred_element_type=jnp.float32)
    return jax.nn.relu(y + b[None, :, None, None])


def _lin(x, w, b):
    y = jnp.matmul(x.astype(jnp.bfloat16), w.T.astype(jnp.bfloat16),
                   preferred_element_type=jnp.float32)
    return y + b


def _forward(obs, conv1_w, conv1_b, conv2_w, conv2_b, conv3_w, conv3_b,
             fc_w, fc_b, u1_w, u1_b, u2_w, u2_b, t1_w, t1_b, t2_w, t2_b,
             actor_w, actor_b, critic_w, critic_b):
    grid, unit = _preprocess(obs)
    x = _conv(grid, conv1_w, conv1_b)
    x = _conv(x, conv2_w, conv2_b)
    x = _conv(x, conv3_w, conv3_b)
    Bn = x.shape[0]
    x = x.reshape(Bn, 64, 4, 4, 4, 4).mean(axis=(3, 5))
    x = x.reshape(Bn, 64 * 4 * 4)
    cnn_feat = jax.nn.relu(_lin(x, fc_w, fc_b))
    uh = jax.nn.relu(_lin(unit, u1_w, u1_b))
    unit_feat = jax.nn.relu(_lin(uh, u2_w, u2_b))
    t = jnp.concatenate([cnn_feat, unit_feat], axis=-1)
    t = jax.nn.relu(_lin(t, t1_w, t1_b))
    t = jax.nn.relu(_lin(t, t2_w, t2_b))
    logits = _lin(t, actor_w, actor_b)
    value = _lin(t, critic_w, critic_b)
    return logits, value


_WEIGHT_KEYS = [
    "conv1_w", "conv1_b", "conv2_w", "conv2_b", "conv3_w", "conv3_b",
    "fc_w", "fc_b", "u1_w", "u1_b", "u2_w", "u2_b", "t1_w", "t1_b",
    "t2_w", "t2_b", "actor_w", "actor_b", "critic_w", "critic_b",
]

_pmapped = None


def _get_pmapped():
    global _pmapped
    if _pmapped is None:
        devs = jax.devices()[:N_CORES]
        _pmapped = jax.pmap(
            _forward,
            in_axes=(0,) + (None,) * len(_WEIGHT_KEYS),
            devices=devs,
        )
    return _pmapped


def kernel(**inputs):
    obs = np.asarray(inputs["obs"], dtype=np.float32)
    B = obs.shape[0]
    shard = B // N_CORES
    obs_sharded = obs.reshape(N_CORES, shard, obs.shape[1])
    weights = [np.asarray(inputs[k], dtype=np.float32) for k in _WEIGHT_KEYS]
    fn = _get_pmapped()
    logits, value = fn(obs_sharded, *weights)
    logits = np.asarray(logits).reshape(B, -1)
    value = np.asarray(value).reshape(B, -1)
    return logits.astype(np.float32), value.astype(np.float32)
